# revision 11
# baseline (speedup 1.0000x reference)
"""BiLSTM-CRF loss kernel for Trainium2 (8 NeuronCores, SPMD data-parallel).

Strategy
--------
Data-parallel over batch: each of 8 cores handles 32 sentences.

Host-side (numpy, cheap):
  * Premultiply the embedding table into per-direction "xg tables"
    E_d[v] = emb[v] @ W_ih_d^T + (b_ih_d + b_hh_d), gate-reordered to
    [i, f, o, g], each gate 128-padded, g-gate pre-scaled by 2 (for the
    tanh(x) = 2*sigmoid(2x) - 1 trick). bf16, [8000, 512].
  * Gather indices (int16, dma_gather layout), one-hot tag mask,
    exp(transitions) matrices for the exp-domain CRF, per-sentence
    host-computable score terms (transition scores, b_out at tags).

Device (Bass/Tile, per core):
  * dma_gather(transpose=True) streams xg^T tiles [128part=4gates*128pad,
    steps*32] straight from the premultiplied tables (fwd ascending,
    bwd descending step order), chunked + double buffered.
  * LSTM: transposed state layout [75, 32] per direction; per step:
    1 identity matmul (xg -> PSUM) + 4 gate matmuls (W_hh^T blocks),
    one fused sigmoid over all 4 gate blocks (g pre-scaled by 2),
    DVE cell update, tanh(c), h write (bf16) into the h-history.
    Two independent chains (fwd/bwd) pipeline across engines.
  * feats^T = w_out_f @ hf + w_out_b @ hb accumulated in PSUM [6, 512]
    chunks; exp(feats + b_out - kappa) -> ef (bf16) for the CRF; the raw
    PSUM chunk also produces the gold emission partials via one-hot
    multiply + segmented reduce.
  * CRF forward pass in the exp domain, q layout [6 states, 32 batch]:
    q <- (exp(trans) @ q) * ef_s  (tiny matmul + DVE multiply per step),
    kappa-centered, renormalized every few steps via a ones-matmul
    column sum + broadcast matmul (no partition reduction needed).
  * Output [3, 32]: log(wstop . q) , accumulated log norms, emission sum.

Host combines: loss = (r0 + r1 + S*kappa) - (r2 + host_terms).
"""
import sys
sys.path.insert(0, "/opt/trn_rl_repo")
from contextlib import ExitStack

import numpy as np
import ml_dtypes

import concourse.bass as bass
import concourse.tile as tile
from concourse import mybir

VOCAB, E, H, T = 8000, 300, 75, 6
B, S = 256, 512
START, STOP = 4, 5
N_CORES = 8
BL = B // N_CORES          # 32 sentences per core
KAPPA = 2.5                # exp-domain centering constant
RENORM = 8                 # CRF renormalization interval (steps)
GCH = 32                   # gather chunk: steps per dma_gather call
FP32 = mybir.dt.float32
BF16 = mybir.dt.bfloat16
AF = mybir.ActivationFunctionType
ALU = mybir.AluOpType

_CTRL_TYPES = ("InstDrain", "InstNop", "InstEventSemaphore")


def _split_excess_waits(nc, limit=1):
    """This walrus build allows only 1 sync-wait per instruction; hoist
    extras onto injected same-engine Drains (same blocking semantics)."""
    f = nc.m.functions[0]
    ctr = 0
    for bb in f.blocks:
        out, changed = [], False
        for inst in bb.instructions:
            si = inst.sync_info
            if si is not None and si.on_wait is not None and len(si.on_wait) > limit:
                waits = list(si.on_wait)
                while len(waits) > limit:
                    chunk, waits = waits[:1], waits[1:]
                    w = mybir.InstDrain(name=f"WSPLIT-{ctr}")
                    ctr += 1
                    w.engine = inst.engine
                    w.sync_info = mybir.SyncInfo(on_wait=chunk, on_update=[])
                    out.append(w)
                    changed = True
                inst.sync_info = mybir.SyncInfo(
                    on_wait=waits, on_update=list(si.on_update or []))
            out.append(inst)
        if changed:
            bb.instructions = out
    return ctr


def build_program(split_waits=True, isa_codegen=True):
    nc = bass.Bass(num_swdge_queues=2)
    NST = S // GCH  # gather chunks per direction

    tab_f = nc.declare_dram_parameter("tab_f", [VOCAB, 512], BF16, isOutput=False)
    tab_b = nc.declare_dram_parameter("tab_b", [VOCAB, 512], BF16, isOutput=False)
    idx_f = nc.declare_dram_parameter("idx_f", [128, S * BL // 16], mybir.dt.int16, isOutput=False)
    idx_b = nc.declare_dram_parameter("idx_b", [128, S * BL // 16], mybir.dt.int16, isOutput=False)
    whhT_f = nc.declare_dram_parameter("whhT_f", [H, 4 * H], BF16, isOutput=False)
    whhT_b = nc.declare_dram_parameter("whhT_b", [H, 4 * H], BF16, isOutput=False)
    identp = nc.declare_dram_parameter("identp", [H, H], BF16, isOutput=False)
    h0T = nc.declare_dram_parameter("h0T", [H, 2 * BL], BF16, isOutput=False)
    c0T = nc.declare_dram_parameter("c0T", [H, 2 * BL], FP32, isOutput=False)
    woutT_f = nc.declare_dram_parameter("woutT_f", [H, T], BF16, isOutput=False)
    woutT_b = nc.declare_dram_parameter("woutT_b", [H, T], BF16, isOutput=False)
    ohT = nc.declare_dram_parameter("ohT", [T, S * BL], BF16, isOutput=False)
    crfW = nc.declare_dram_parameter("crfW", [T, T], FP32, isOutput=False)
    wstop = nc.declare_dram_parameter("wstop", [T, 1], FP32, isOutput=False)
    onesT = nc.declare_dram_parameter("onesT", [T, 1], FP32, isOutput=False)
    ones1 = nc.declare_dram_parameter("ones1", [1, T], FP32, isOutput=False)
    q0 = nc.declare_dram_parameter("q0", [T, BL], FP32, isOutput=False)
    bias_bk = nc.declare_dram_parameter("bias_bk", [T, 1], FP32, isOutput=False)
    dev_out = nc.declare_dram_parameter("dev_out", [3, BL], FP32, isOutput=True)

    from concourse import library_config

    with tile.TileContext(nc) as tc, ExitStack() as ctx:
        nc.gpsimd.load_library(library_config.mlp)  # DMAGatherAnt lives here
        const = ctx.enter_context(tc.tile_pool(name="const", bufs=1))
        hist = ctx.enter_context(tc.tile_pool(name="hist", bufs=1))
        state = ctx.enter_context(tc.tile_pool(name="state", bufs=1))
        efp = ctx.enter_context(tc.tile_pool(name="efp", bufs=1))
        outp = ctx.enter_context(tc.tile_pool(name="outp", bufs=1))

        # ---- constants to SBUF ----
        whh, wout = {}, {}
        for d, srcw, srco in (("f", whhT_f, woutT_f), ("b", whhT_b, woutT_b)):
            t = const.tile([H, 4 * H], BF16, name=f"whh_{d}")
            nc.sync.dma_start(t[:], srcw[:]); whh[d] = t
            t = const.tile([H, T], BF16, name=f"wout_{d}")
            nc.sync.dma_start(t[:], srco[:]); wout[d] = t
        idt = const.tile([H, H], BF16, name="idt"); nc.sync.dma_start(idt[:], identp[:])
        idxs = {}
        for d, src in (("f", idx_f), ("b", idx_b)):
            t = const.tile([128, S * BL // 16], mybir.dt.int16, name=f"idx_{d}")
            nc.sync.dma_start(t[:], src[:]); idxs[d] = t
        oh = const.tile([T, S * BL], BF16, name="oh"); nc.sync.dma_start(oh[:], ohT[:])
        cW = const.tile([T, T], FP32, name="cW"); nc.sync.dma_start(cW[:], crfW[:])
        ws = const.tile([T, 1], FP32, name="ws"); nc.sync.dma_start(ws[:], wstop[:])
        o6 = const.tile([T, 1], FP32, name="o6"); nc.sync.dma_start(o6[:], onesT[:])
        o1 = const.tile([1, T], FP32, name="o1"); nc.sync.dma_start(o1[:], ones1[:])
        bbk = const.tile([T, 1], FP32, name="bbk"); nc.sync.dma_start(bbk[:], bias_bk[:])

        hT, cst = {}, {}
        for di, d in enumerate("fb"):
            t = hist.tile([H, (S + 1) * BL], BF16, name=f"hT_{d}")
            hT[d] = t
            t2 = state.tile([H, BL], FP32, name=f"c_{d}")
            nc.sync.dma_start(t2[:], c0T[:, di * BL:(di + 1) * BL]); cst[d] = t2
        # h0 slots: fwd -> time-col 0; bwd -> time-col S (see indexing below)
        nc.sync.dma_start(hT["f"][:, 0:BL], h0T[:, 0:BL])
        nc.sync.dma_start(hT["b"][:, S * BL:(S + 1) * BL], h0T[:, BL:2 * BL])

        ef = efp.tile([T, S * BL], BF16, name="ef")
        r0t = outp.tile([1, BL], FP32, name="r0t")
        r2t = outp.tile([1, BL], FP32, name="r2t")
        logacc = outp.tile([1, BL], FP32, name="logacc")
        nc.vector.memset(logacc[:], 0.0)

        # ================= gather + LSTM =================
        with tc.tile_pool(name="xgp", bufs=2) as xgp, \
             tc.tile_pool(name="work", bufs=3) as work, \
             tc.tile_pool(name="psumL", bufs=4, space="PSUM") as psumL:

            xgt = {}

            def gather(d, ch):
                t = xgp.tile([128, 4, GCH * BL], BF16, name=f"xg_{d}", tag=f"xg_{d}")
                tab = tab_f if d == "f" else tab_b
                nc.gpsimd.dma_gather(
                    t[:], tab[:], idxs[d][:, ch * GCH * BL // 16:(ch + 1) * GCH * BL // 16],
                    num_idxs=GCH * BL, num_idxs_reg=GCH * BL,
                    elem_size=512, transpose=True, single_packet=False,
                    queue_num=0 if d == "f" else 1)
                xgt[d] = t

            def step(sc, d):
                # sc: chain-local step. fwd: time t=sc, reads hcol t, writes t+1.
                # bwd: time t=S-1-sc, reads hcol t+1, writes t.
                if d == "f":
                    rd, wr = sc, sc + 1
                else:
                    rd, wr = S - sc, S - 1 - sc
                G = psumL.tile([H, 4 * BL], FP32, name=f"G_{d}", tag=f"G_{d}")
                xsl = xgt[d][0:H, :, (sc % GCH) * BL:((sc % GCH) + 1) * BL]
                nc.tensor.matmul(G[:], idt[:], xsl, start=True, stop=False)
                hprev = hT[d][:, rd * BL:(rd + 1) * BL]
                for gi in range(4):
                    nc.tensor.matmul(G[:, gi * BL:(gi + 1) * BL],
                                     whh[d][:, gi * H:(gi + 1) * H], hprev,
                                     start=False, stop=(gi == 3))
                SIG = work.tile([H, 4 * BL], FP32, name=f"SIG_{d}", tag=f"SIG_{d}")
                nc.scalar.activation(SIG[:], G[:], AF.Sigmoid)
                si, sf, so, s2g = (SIG[:, k * BL:(k + 1) * BL] for k in range(4))
                t2 = work.tile([H, BL], FP32, name=f"t2_{d}", tag=f"t2_{d}")
                nc.vector.tensor_mul(t2[:], sf, cst[d][:])
                A = work.tile([H, BL], FP32, name=f"A_{d}", tag=f"A_{d}")
                nc.vector.scalar_tensor_tensor(A[:], s2g, 2.0, si,
                                               op0=ALU.mult, op1=ALU.mult)
                Bt = work.tile([H, BL], FP32, name=f"B_{d}", tag=f"B_{d}")
                nc.vector.tensor_sub(Bt[:], A[:], si)
                nc.vector.tensor_add(cst[d][:], Bt[:], t2[:])
                TC = work.tile([H, BL], FP32, name=f"TC_{d}", tag=f"TC_{d}")
                nc.scalar.activation(TC[:], cst[d][:], AF.Tanh)
                nc.vector.tensor_mul(hT[d][:, wr * BL:(wr + 1) * BL], so, TC[:])

            for ch in range(S // GCH):
                gather("f", ch)
                gather("b", ch)
                for s0 in range(GCH):
                    sc = ch * GCH + s0
                    step(sc, "f")
                    step(sc, "b")

        # ================= feats + emission + ef =================
        FCH = 512  # feats chunk: columns of feats^T per matmul (16 steps)
        nch = S * BL // FCH
        with tc.tile_pool(name="emp", bufs=1) as emp, \
             tc.tile_pool(name="workF", bufs=3) as workF, \
             tc.tile_pool(name="psumF", bufs=4, space="PSUM") as psumF:
            empart = emp.tile([T, nch * BL], FP32, name="empart")
            for k in range(nch):
                FT = psumF.tile([T, FCH], FP32, name="FT", tag="FT")
                # hf[t] at hcol t+1 ; hb[t] at hcol t  (t = time)
                hfs = hT["f"][:, k * FCH + BL:(k + 1) * FCH + BL]
                hbs = hT["b"][:, k * FCH:(k + 1) * FCH]
                nc.tensor.matmul(FT[:], wout["f"][:], hfs, start=True, stop=False)
                nc.tensor.matmul(FT[:], wout["b"][:], hbs, start=False, stop=True)
                # ef chunk = exp(feats + b_out - kappa)
                nc.scalar.activation(ef[:, k * FCH:(k + 1) * FCH], FT[:],
                                     AF.Exp, bias=bbk[:, 0:1])
                # emission partials: mask by one-hot, reduce over steps in chunk
                EMM = workF.tile([T, FCH], FP32, name="EMM", tag="EMM")
                nc.vector.tensor_mul(EMM[:], FT[:], oh[:, k * FCH:(k + 1) * FCH])
                # view [T, b(32) x s(FCH/BL)]: col = s*BL + b -> reduce over s
                emv = EMM[:].rearrange("p (s b) -> p b s", b=BL)
                nc.vector.tensor_reduce(
                    empart[:, k * BL:(k + 1) * BL], emv, op=ALU.add,
                    axis=mybir.AxisListType.X)
            # total emission: reduce partials over chunks, then contract T via matmul
            emtot = workF.tile([T, BL], FP32, name="emtot")
            nc.vector.tensor_reduce(
                emtot[:], empart[:].rearrange("p (k b) -> p b k", b=BL),
                op=ALU.add, axis=mybir.AxisListType.X)
            # [1, BL] = ones6.T @ emtot  (contracts the T partitions)
            EMP = psumF.tile([1, BL], FP32, name="EMP", tag="EMP")
            nc.tensor.matmul(EMP[:], o6[:], emtot[:], start=True, stop=True)
            nc.scalar.copy(r2t[:], EMP[:])

        # ================= CRF =================
        with tc.tile_pool(name="qp", bufs=1) as qpool, \
             tc.tile_pool(name="workC", bufs=3) as workC, \
             tc.tile_pool(name="psumC", bufs=2, space="PSUM") as psumC:
            q = qpool.tile([T, BL], FP32, name="q")
            nc.sync.dma_start(q[:], q0[:])
            for s in range(S):
                QP = psumC.tile([T, BL], FP32, name="QP", tag="QP")
                nc.tensor.matmul(QP[:], cW[:], q[:], start=True, stop=True)
                nc.vector.tensor_mul(q[:], QP[:], ef[:, s * BL:(s + 1) * BL])
                if (s + 1) % RENORM == 0:
                    SM = psumC.tile([1, BL], FP32, name="SM", tag="SM")
                    nc.tensor.matmul(SM[:], o6[:], q[:], start=True, stop=True)
                    rc = workC.tile([1, BL], FP32, name="rc", tag="rc")
                    nc.vector.reciprocal(rc[:], SM[:])
                    RB = psumC.tile([T, BL], FP32, name="RB", tag="RB")
                    nc.tensor.matmul(RB[:], o1[:], rc[:], start=True, stop=True)
                    nc.vector.tensor_mul(q[:], q[:], RB[:])
                    lg = workC.tile([1, BL], FP32, name="lg", tag="lg")
                    nc.scalar.activation(lg[:], SM[:], AF.Ln)
                    nc.vector.tensor_add(logacc[:], logacc[:], lg[:])
            # final: r0 = log(wstop . q)
            FS = psumC.tile([1, BL], FP32, name="FS", tag="FS")
            nc.tensor.matmul(FS[:], ws[:], q[:], start=True, stop=True)
            nc.scalar.activation(r0t[:], FS[:], AF.Ln)

        nc.sync.dma_start(dev_out[0:1, :], r0t[:])
        nc.sync.dma_start(dev_out[1:2, :], logacc[:])
        nc.sync.dma_start(dev_out[2:3, :], r2t[:])

    if isa_codegen:
        mybir.codegen_inst_isa_subclasses(nc)
    if split_waits:
        _split_excess_waits(nc)
    return nc


# ---------------- host side ----------------

def _prep_core_inputs(inputs, core):
    f32 = np.float32
    bf = ml_dtypes.bfloat16
    sl = slice(core * BL, (core + 1) * BL)
    sent = np.asarray(inputs["sentence"])[sl]          # [BL, S]
    tags = np.asarray(inputs["tags"])[sl]
    trans = np.asarray(inputs["transitions"], f32)
    b_out = np.asarray(inputs["b_out"], f32)

    d = {}
    d["idx_f"] = np.ascontiguousarray(np.tile(
        sent.T.reshape(-1).astype(np.int16).reshape(-1, 16).T, (8, 1)))
    d["idx_b"] = np.ascontiguousarray(np.tile(
        sent.T[::-1].reshape(-1).astype(np.int16).reshape(-1, 16).T, (8, 1)))
    h0 = np.asarray(inputs["h0"], f32)[:, sl]          # [2, BL, H]
    c0 = np.asarray(inputs["c0"], f32)[:, sl]
    d["h0T"] = np.concatenate([h0[0].T, h0[1].T], axis=1).astype(bf)
    d["c0T"] = np.concatenate([c0[0].T, c0[1].T], axis=1).astype(f32)
    # one-hot tags^T [T, S*BL] (col = s*BL + b)
    ohT = np.zeros((T, S * BL), f32)
    cols = np.arange(S * BL)
    ohT[tags.T.reshape(-1), cols] = 1.0
    d["ohT"] = ohT.astype(bf)
    d["crfW"] = np.exp(trans).T.copy().astype(f32)     # lhsT: [i, j] = exp(trans[j, i])
    d["wstop"] = np.exp(trans[STOP])[:, None].astype(f32)
    d["onesT"] = np.ones((T, 1), f32)
    d["ones1"] = np.ones((1, T), f32)
    q0 = np.zeros((T, BL), f32); q0[START] = 1.0
    d["q0"] = q0
    d["bias_bk"] = (b_out - KAPPA)[:, None].astype(f32)
    # host-computable score pieces
    tags_ext = np.concatenate([np.full((BL, 1), START, tags.dtype), tags], axis=1)
    trans_sc = trans[tags_ext[:, 1:], tags_ext[:, :-1]].sum(axis=1)
    host_corr = (trans_sc + b_out[tags].sum(axis=1)
                 + trans[STOP, tags[:, -1]]).astype(f32)
    return d, host_corr


def _prep_shared(inputs):
    f32 = np.float32
    bf = ml_dtypes.bfloat16
    emb = np.asarray(inputs["embedding"], f32)
    d = {}
    # gate reorder [i, f, o, g] from pytorch [i, f, g, o]; g pre-scaled by 2
    perm = [0, 1, 3, 2]  # new block k takes old gate perm[k]
    for dd in "fb":
        w_ih = np.asarray(inputs[f"w_ih_{dd}"], f32)    # [4H, E]
        w_hh = np.asarray(inputs[f"w_hh_{dd}"], f32)    # [4H, H]
        bias = (np.asarray(inputs[f"b_ih_{dd}"], f32)
                + np.asarray(inputs[f"b_hh_{dd}"], f32))
        tab = np.zeros((VOCAB, 4, 128), f32)
        whhT = np.zeros((H, 4 * H), f32)
        for k in range(4):
            g = perm[k]
            scale = 2.0 if g == 2 else 1.0
            rows = slice(g * H, (g + 1) * H)
            tab[:, k, 0:H] = scale * (emb @ w_ih[rows].T + bias[rows])
            whhT[:, k * H:(k + 1) * H] = scale * w_hh[rows].T
        d[f"tab_{dd}"] = tab.reshape(VOCAB, 512).astype(bf)
        d[f"whhT_{dd}"] = whhT.astype(bf)
    w_out = np.asarray(inputs["w_out"], f32)            # [T, 2H]
    d["woutT_f"] = w_out[:, 0:H].T.copy().astype(bf)
    d["woutT_b"] = w_out[:, H:2 * H].T.copy().astype(bf)
    d["identp"] = np.eye(H).astype(bf)
    return d


_RUNNER = None


def _get_runner():
    global _RUNNER
    if _RUNNER is None:
        import jax
        from jax.sharding import Mesh, PartitionSpec
        from jax.experimental.shard_map import shard_map
        from concourse.bass2jax import (_bass_exec_p, install_neuronx_cc_hook,
                                        partition_id_tensor)
        install_neuronx_cc_hook()
        nc = build_program()
        partition_name = (nc.partition_id_tensor.name
                          if nc.partition_id_tensor else None)
        in_names, out_names, out_avals, zero_outs = [], [], [], []
        for alloc in nc.m.functions[0].allocations:
            if not isinstance(alloc, mybir.MemoryLocationSet):
                continue
            name = alloc.memorylocations[0].name
            if alloc.kind == "ExternalInput":
                if name != partition_name:
                    in_names.append(name)
            elif alloc.kind == "ExternalOutput":
                out_names.append(name)
                shape = tuple(alloc.tensor_shape)
                dtype = mybir.dt.np(alloc.dtype)
                out_avals.append(jax.core.ShapedArray(shape, dtype))
                zero_outs.append(np.zeros(shape, dtype))
        n_params = len(in_names)
        all_in = list(in_names) + list(out_names)
        if partition_name is not None:
            all_in.append(partition_name)

        def _body(*args):
            operands = list(args)
            if partition_name is not None:
                operands.append(partition_id_tensor())
            return tuple(_bass_exec_p.bind(
                *operands, out_avals=tuple(out_avals), in_names=tuple(all_in),
                out_names=tuple(out_names), lowering_input_output_aliases=(),
                sim_require_finite=True, sim_require_nnan=True, nc=nc))

        donate = tuple(range(n_params, n_params + len(out_avals)))
        devices = jax.devices()[:N_CORES]
        mesh = Mesh(np.asarray(devices), ("core",))
        fn = jax.jit(
            shard_map(_body, mesh=mesh,
                      in_specs=(PartitionSpec("core"),) * (n_params + len(out_avals)),
                      out_specs=(PartitionSpec("core"),) * len(out_avals),
                      check_rep=False),
            donate_argnums=donate, keep_unused=True)
        _RUNNER = (nc, fn, in_names, out_names, out_avals, zero_outs)
    return _RUNNER


def kernel(**inputs):
    nc, fn, in_names, out_names, out_avals, zero_outs = _get_runner()
    shared = _prep_shared(inputs)
    per_core, host_corr = [], []
    for c in range(N_CORES):
        d, hc = _prep_core_inputs(inputs, c)
        d.update(shared)
        per_core.append(d)
        host_corr.append(hc)
    concat_in = [np.concatenate([per_core[c][n] for c in range(N_CORES)], axis=0)
                 for n in in_names]
    concat_zeros = [np.zeros((N_CORES * z.shape[0], *z.shape[1:]), z.dtype)
                    for z in zero_outs]
    outs = fn(*concat_in, *concat_zeros)
    res = np.asarray(outs[0]).reshape(N_CORES, 3, BL)
    loss = np.empty(B, np.float32)
    for c in range(N_CORES):
        r0, r1, r2 = res[c]
        loss[c * BL:(c + 1) * BL] = (r0 + r1 + S * KAPPA) - (r2 + host_corr[c])
    return loss
